# revision 2
# baseline (speedup 1.0000x reference)
"""AUGRU (VecAttGRUCell) dynamic_rnn kernel for Trainium2, 8 NeuronCores.

Problem: B=1024, T=512, D=128 (fp32).
    gi = [x, h] @ gate_kernel + gate_bias ; r, u = split(sigmoid(gi))
    c  = tanh([x, r*h] @ cand_kernel + cand_bias)
    u' = (1 - att) * u
    h' = u'*h + (1-u')*c            (h' = c + u'*(h-c))
    out[t] = h' masked to 0 for t >= len ; h frozen past len.

Key host-side simplifications (device runs the UNMASKED recurrence):
  - out[b, t] for t < len[b] only depends on the unmasked recurrence;
    for t >= len[b] the reference output is 0 -> host zeroes it after.
  - alpha = (1 - att) precomputed on host, broadcast on-device via a
    rank-1 matmul (ones[1,128] x alpha_row[1,128] -> PSUM tile).

Sharding: batch 1024 -> 8 cores x 128 rows. Per core everything is
feature-major [d, b] / [j, b]; 128 batch rows sit on the free axis and
128 features on partitions, so the weights load as stationary [d, j]
blocks straight from HBM with no transposes anywhere on device.
Host pre-transposes X to [D, T, B_sh] per core and post-transposes the
[T, D, B_sh] output back to [B, T, D].
"""

import numpy as np

import concourse.bacc as bacc
import concourse.mybir as mybir
import concourse.tile as tile
import concourse.bass as bass
from concourse.bass_utils import run_bass_kernel_spmd

F32 = mybir.dt.float32
AF = mybir.ActivationFunctionType

B, T, D = 1024, 512, 128
NCORES = 8
BSH = B // NCORES          # batch rows per core = 128
CHUNK = 32                 # timesteps per DMA chunk

_module_cache = {}


def build_module(t_steps: int = T, chunk: int = CHUNK):
    """Build + finalize the Bacc module for one core (SPMD across 8)."""
    key = (t_steps, chunk)
    if key in _module_cache:
        return _module_cache[key]
    assert t_steps % chunk == 0
    nchunks = t_steps // chunk

    nc = bacc.Bacc("TRN2", target_bir_lowering=False)

    # DRAM I/O (per-core shapes)
    X = nc.dram_tensor("X", (D, t_steps, BSH), F32, kind="ExternalInput")
    A = nc.dram_tensor("A", (1, t_steps * BSH), F32, kind="ExternalInput")
    GK = nc.dram_tensor("GK", (2 * D, 2 * D), F32, kind="ExternalInput")
    CK = nc.dram_tensor("CK", (2 * D, D), F32, kind="ExternalInput")
    GB = nc.dram_tensor("GB", (1, 2 * D), F32, kind="ExternalInput")
    CB = nc.dram_tensor("CB", (1, D), F32, kind="ExternalInput")
    OUT = nc.dram_tensor("OUT", (t_steps, D, BSH), F32, kind="ExternalOutput")

    with tile.TileContext(nc) as tc:
        with (
            tc.tile_pool(name="const", bufs=1) as constp,
            tc.tile_pool(name="xch", bufs=2) as xpool,
            tc.tile_pool(name="ach", bufs=2) as apool,
            tc.tile_pool(name="state", bufs=2) as hpool,
            tc.tile_pool(name="work", bufs=3) as wpool,
            tc.tile_pool(name="pru", bufs=2, space="PSUM") as pru_pool,
            tc.tile_pool(name="pc", bufs=2, space="PSUM") as pc_pool,
            tc.tile_pool(name="pa", bufs=2, space="PSUM") as pa_pool,
        ):
            # --- constants: weights, biases, ones --------------------------
            wxr = constp.tile([D, D], F32, tag="wxr")
            wxu = constp.tile([D, D], F32, tag="wxu")
            whr = constp.tile([D, D], F32, tag="whr")
            whu = constp.tile([D, D], F32, tag="whu")
            cx = constp.tile([D, D], F32, tag="cx")
            ch = constp.tile([D, D], F32, tag="ch")
            gbr = constp.tile([1, D], F32, tag="gbr")
            gbu = constp.tile([1, D], F32, tag="gbu")
            cb1 = constp.tile([1, D], F32, tag="cb1")
            ones = constp.tile([1, D], F32, tag="ones")

            nc.sync.dma_start(wxr[:], GK[0:D, 0:D])
            nc.sync.dma_start(wxu[:], GK[0:D, D : 2 * D])
            nc.sync.dma_start(whr[:], GK[D : 2 * D, 0:D])
            nc.sync.dma_start(whu[:], GK[D : 2 * D, D : 2 * D])
            nc.sync.dma_start(cx[:], CK[0:D, :])
            nc.sync.dma_start(ch[:], CK[D : 2 * D, :])
            nc.sync.dma_start(gbr[:], GB[0:1, 0:D])
            nc.sync.dma_start(gbu[:], GB[0:1, D : 2 * D])
            nc.sync.dma_start(cb1[:], CB[:])
            nc.gpsimd.memset(ones[:], 1.0)

            # --- initial state --------------------------------------------
            h_cur = hpool.tile([D, BSH], F32, tag="h")
            nc.gpsimd.memset(h_cur[:], 0.0)

            for ci in range(nchunks):
                c0 = ci * chunk
                xch = xpool.tile([D, chunk, BSH], F32, tag="xch")
                nc.sync.dma_start(xch[:], X[:, c0 : c0 + chunk, :])
                ach = apool.tile([1, chunk * BSH], F32, tag="ach")
                nc.sync.dma_start(ach[:], A[0:1, c0 * BSH : (c0 + chunk) * BSH])

                for i in range(chunk):
                    t = c0 + i
                    x_t = xch[:, i, :]

                    # alpha broadcast: pa[j, b] = alpha_t[b]
                    pa = pa_pool.tile([D, BSH], F32, tag="pa")
                    nc.tensor.matmul(
                        pa[:], ones[:], ach[0:1, bass.ts(i, BSH)],
                        start=True, stop=True,
                    )

                    # gate pre-activations [j, b]: r in cols of slice 0, u in slice 1
                    pru = pru_pool.tile([D, 2, BSH], F32, tag="pru")
                    nc.tensor.matmul(pru[:, 0, :], wxr[:], x_t, start=True, stop=False)
                    nc.tensor.matmul(pru[:, 0, :], gbr[:], ones[:], start=False, stop=False)
                    nc.tensor.matmul(pru[:, 0, :], whr[:], h_cur[:], start=False, stop=True)
                    nc.tensor.matmul(pru[:, 1, :], wxu[:], x_t, start=True, stop=False)
                    nc.tensor.matmul(pru[:, 1, :], gbu[:], ones[:], start=False, stop=False)
                    nc.tensor.matmul(pru[:, 1, :], whu[:], h_cur[:], start=False, stop=True)

                    ru = wpool.tile([D, 2, BSH], F32, tag="ru")
                    nc.scalar.activation(ru[:], pru[:], AF.Sigmoid)

                    rh = wpool.tile([D, BSH], F32, tag="rh")
                    nc.vector.tensor_mul(rh[:], ru[:, 0, :], h_cur[:])

                    # candidate pre-activation
                    pc = pc_pool.tile([D, BSH], F32, tag="pc")
                    nc.tensor.matmul(pc[:], cx[:], x_t, start=True, stop=False)
                    nc.tensor.matmul(pc[:], cb1[:], ones[:], start=False, stop=False)
                    nc.tensor.matmul(pc[:], ch[:], rh[:], start=False, stop=True)

                    c_t = wpool.tile([D, BSH], F32, tag="c")
                    nc.scalar.activation(c_t[:], pc[:], AF.Tanh)

                    # z = alpha * u ; h' = c + z*(h-c)
                    z = wpool.tile([D, BSH], F32, tag="z")
                    nc.vector.tensor_mul(z[:], ru[:, 1, :], pa[:])
                    dd = wpool.tile([D, BSH], F32, tag="dd")
                    nc.vector.tensor_sub(dd[:], h_cur[:], c_t[:])
                    ee = wpool.tile([D, BSH], F32, tag="ee")
                    nc.vector.tensor_mul(ee[:], z[:], dd[:])
                    h_new = hpool.tile([D, BSH], F32, tag="h")
                    nc.vector.tensor_add(h_new[:], c_t[:], ee[:])

                    nc.sync.dma_start(OUT[t, :, :], h_new[:])
                    h_cur = h_new

    nc.finalize()
    _module_cache[key] = nc
    return nc


def kernel(rnn_input, att_score, gate_kernel, gate_bias, cand_kernel,
           cand_bias, sequence_length, _t_steps: int = T):
    """Full-input entry point: shard across 8 cores, run, unshard."""
    t_steps = _t_steps
    rnn_input = np.ascontiguousarray(np.asarray(rnn_input, dtype=np.float32))
    att_score = np.asarray(att_score, dtype=np.float32)
    gate_kernel = np.ascontiguousarray(np.asarray(gate_kernel, dtype=np.float32))
    gate_bias = np.asarray(gate_bias, dtype=np.float32).reshape(1, 2 * D)
    cand_kernel = np.ascontiguousarray(np.asarray(cand_kernel, dtype=np.float32))
    cand_bias = np.asarray(cand_bias, dtype=np.float32).reshape(1, D)
    lens = np.asarray(sequence_length, dtype=np.int32).reshape(-1)

    nc = build_module(t_steps)

    in_maps = []
    for cid in range(NCORES):
        sl = slice(cid * BSH, (cid + 1) * BSH)
        xs = rnn_input[sl, :t_steps, :]                       # [BSH, t, D]
        Xc = np.ascontiguousarray(np.transpose(xs, (2, 1, 0)))  # [D, t, BSH]
        al = 1.0 - att_score[sl, :t_steps, 0]                 # [BSH, t]
        Ac = np.ascontiguousarray(al.T).reshape(1, t_steps * BSH)
        in_maps.append({
            "X": Xc, "A": Ac,
            "GK": gate_kernel, "CK": cand_kernel,
            "GB": gate_bias, "CB": cand_bias,
        })

    res = run_bass_kernel_spmd(nc, in_maps, list(range(NCORES)))

    out = np.empty((B, t_steps, D), dtype=np.float32)
    for cid in range(NCORES):
        oc = res.results[cid]["OUT"]                          # [t, D, BSH]
        out[cid * BSH : (cid + 1) * BSH] = np.transpose(oc, (2, 0, 1))

    # dynamic_rnn zeroing past each sequence length
    tmask = np.arange(t_steps)[None, :] >= np.minimum(lens, t_steps)[:, None]
    out[tmask] = 0.0
    return out


# revision 10
# speedup vs baseline: 7.2692x; 7.2692x over previous
"""AUGRU (VecAttGRUCell) dynamic_rnn kernel for Trainium2, 8 NeuronCores.

Problem: B=1024, T=512, D=128 (fp32).
    gi = [x, h] @ gate_kernel + gate_bias ; r, u = split(sigmoid(gi))
    c  = tanh([x, r*h] @ cand_kernel + cand_bias)
    u' = (1 - att) * u ; h' = u'*h + (1-u')*c
    out[t] = h' for t < len, else 0 ; h frozen past len.

Device runs the UNMASKED recurrence (outputs for t < len only depend on
it; host zeroes t >= len afterwards). alpha = (1 - att) is precomputed
on host and broadcast on-device with a rank-1 matmul.

Sharding: batch 1024 -> 8 cores x 128 rows. Everything on device is
feature-major [d, b]: batch on the free axis, features on partitions,
weights loaded as stationary [d, j] blocks with no device transposes.
Host pre-transposes X to [D, T, B_sh] per core, post-transposes the
[T, D, B_sh] output back to [B, T, D].

Per step (the serial h -> h' chain dominates; ~7 engine hops):
  whr MM -> sigma_r (ACT, bias AP) -> rh (DVE) -> ch MM -> tanh (ACT)
  -> g = (z-1)*c (DVE STT) -> h' = p - g (DVE), with the u-path
  (whu MM, sigma_u, z = u*alpha_bcast, p = z*h on GPSIMD) off-chain.
x-projections and the alpha broadcast are batched 4 steps per matmul
into dedicated PSUM banks the h-matmuls then accumulate into.
"""

import numpy as np

import concourse.bacc as bacc
import concourse.mybir as mybir
import concourse.tile as tile
import concourse.bass as bass
from concourse.bass_utils import run_bass_kernel_spmd

F32 = mybir.dt.float32
AF = mybir.ActivationFunctionType
OP = mybir.AluOpType

B, T, D = 1024, 512, 128
NCORES = 8
BSH = B // NCORES          # batch rows per core = 128
CHUNK = 32                 # timesteps per DMA chunk

_module_cache = {}


def _emit_chunk(nc, pools, consts, h_cur, t_base, xch, ach, OUT, chunk,
                dyn=False):
    """Emit one chunk (`chunk` timesteps). t_base is an int (unrolled) or a
    RuntimeValue (For_i). Returns the AP holding the final h."""
    wpool, pru_pool, pc_pool, pa_pool = pools
    wxr, wxu, whr, whu, cx, ch, gbr, gbu, cbc, ones = consts

    for q in range(chunk // 4):
        q0 = q * 4
        pr4 = pru_pool.tile([D, 4, BSH], F32, tag="pr4", name=f"pr4_{q}")
        pu4 = pru_pool.tile([D, 4, BSH], F32, tag="pu4", name=f"pu4_{q}")
        pc4 = pc_pool.tile([D, 4, BSH], F32, tag="pc4", name=f"pc4_{q}")
        pa4 = pa_pool.tile([D, 4, BSH], F32, tag="pa4", name=f"pa4_{q}")
        xq = xch[:, q0 : q0 + 4, :]
        nc.tensor.matmul(pr4[:], wxr[:], xq, start=True, stop=True)
        nc.tensor.matmul(pu4[:], wxu[:], xq, start=True, stop=True)
        nc.tensor.matmul(pc4[:], cx[:], xq, start=True, stop=True)
        nc.tensor.matmul(pa4[:], ones[:], ach[0:1, bass.ts(q, 4 * BSH)],
                         start=True, stop=True)

        for i in range(4):
            t = t_base + q0 + i
            h_c = h_cur
            # --- critical chain ---------------------------------------
            nc.tensor.matmul(pr4[:, i, :], whr[:], h_c,
                             start=False, stop=True, skip_group_check=True)
            r_t = wpool.tile([D, BSH], F32, tag="r", name=f"r_{q}_{i}")
            nc.scalar.activation(r_t[:], pr4[:, i, :], AF.Sigmoid, bias=gbr[:])
            # u-path interleaved so in-order ACT does sigma_u in the gap
            nc.tensor.matmul(pu4[:, i, :], whu[:], h_c,
                             start=False, stop=True, skip_group_check=True)
            u_t = wpool.tile([D, BSH], F32, tag="u", name=f"u_{q}_{i}")
            nc.scalar.activation(u_t[:], pu4[:, i, :], AF.Sigmoid, bias=gbu[:])
            rh = wpool.tile([D, BSH], F32, tag="rh", name=f"rh_{q}_{i}")
            nc.vector.tensor_mul(rh[:], r_t[:], h_c)
            nc.tensor.matmul(pc4[:, i, :], ch[:], rh[:],
                             start=False, stop=True, skip_group_check=True)
            c_t = wpool.tile([D, BSH], F32, tag="c", name=f"c_{q}_{i}")
            nc.scalar.activation(c_t[:], pc4[:, i, :], AF.Tanh, bias=cbc[:])
            # --- off-chain tail ---------------------------------------
            z = wpool.tile([D, BSH], F32, tag="z", name=f"z_{q}_{i}")
            nc.vector.tensor_mul(z[:], u_t[:], pa4[:, i, :])
            p_t = wpool.tile([D, BSH], F32, tag="p", name=f"p_{q}_{i}")
            nc.gpsimd.tensor_mul(p_t[:], z[:], h_c)
            # h' = z*h + (1-z)*c = p - (z-1)*c
            g_t = wpool.tile([D, BSH], F32, tag="g", name=f"g_{q}_{i}")
            nc.vector.scalar_tensor_tensor(g_t[:], z[:], 1.0, c_t[:],
                                           OP.subtract, OP.mult)
            h_new = wpool.tile([D, BSH], F32, tag="h", name=f"h_{q}_{i}")
            nc.vector.tensor_sub(h_new[:], p_t[:], g_t[:])
            if dyn:
                nc.sync.dma_start(OUT[bass.ds(t, 1), :, :], h_new[:])
            else:
                nc.sync.dma_start(OUT[t, :, :], h_new[:])
            h_cur = h_new[:]
    return h_cur


def _build(nc, t_steps, chunk, looped):
    nchunks = t_steps // chunk
    X = nc.dram_tensor("X", (D, t_steps, BSH), F32, kind="ExternalInput")
    A = nc.dram_tensor("A", (1, t_steps * BSH), F32, kind="ExternalInput")
    GK = nc.dram_tensor("GK", (2 * D, 2 * D), F32, kind="ExternalInput")
    CK = nc.dram_tensor("CK", (2 * D, D), F32, kind="ExternalInput")
    GBR = nc.dram_tensor("GBR", (D, 1), F32, kind="ExternalInput")
    GBU = nc.dram_tensor("GBU", (D, 1), F32, kind="ExternalInput")
    CBC = nc.dram_tensor("CBC", (D, 1), F32, kind="ExternalInput")
    OUT = nc.dram_tensor("OUT", (t_steps, D, BSH), F32, kind="ExternalOutput")

    with tile.TileContext(nc) as tc:
        with (
            tc.tile_pool(name="const", bufs=1) as constp,
            tc.tile_pool(name="xch", bufs=2) as xpool,
            tc.tile_pool(name="ach", bufs=2) as apool,
            tc.tile_pool(name="work", bufs=3) as wpool,
            tc.tile_pool(name="pru", bufs=2, space="PSUM") as pru_pool,
            tc.tile_pool(name="pc", bufs=2, space="PSUM") as pc_pool,
            tc.tile_pool(name="pa", bufs=2, space="PSUM") as pa_pool,
        ):
            pools = (wpool, pru_pool, pc_pool, pa_pool)
            wxr = constp.tile([D, D], F32, tag="wxr")
            wxu = constp.tile([D, D], F32, tag="wxu")
            whr = constp.tile([D, D], F32, tag="whr")
            whu = constp.tile([D, D], F32, tag="whu")
            cx = constp.tile([D, D], F32, tag="cx")
            ch = constp.tile([D, D], F32, tag="ch")
            gbr = constp.tile([D, 1], F32, tag="gbr")
            gbu = constp.tile([D, 1], F32, tag="gbu")
            cbc = constp.tile([D, 1], F32, tag="cbc")
            ones = constp.tile([1, D], F32, tag="ones")
            consts = (wxr, wxu, whr, whu, cx, ch, gbr, gbu, cbc, ones)

            nc.sync.dma_start(wxr[:], GK[0:D, 0:D])
            nc.sync.dma_start(wxu[:], GK[0:D, D : 2 * D])
            nc.sync.dma_start(whr[:], GK[D : 2 * D, 0:D])
            nc.sync.dma_start(whu[:], GK[D : 2 * D, D : 2 * D])
            nc.sync.dma_start(cx[:], CK[0:D, :])
            nc.sync.dma_start(ch[:], CK[D : 2 * D, :])
            nc.sync.dma_start(gbr[:], GBR[:])
            nc.sync.dma_start(gbu[:], GBU[:])
            nc.sync.dma_start(cbc[:], CBC[:])
            nc.gpsimd.memset(ones[:], 1.0)

            if looped:
                # fixed-address state tile: each loop iteration starts and
                # ends with h in this tile
                hst = constp.tile([D, BSH], F32, tag="hst", name="h_state")
                nc.gpsimd.memset(hst[:], 0.0)
                with tc.For_i(0, nchunks, 1) as ci:
                    t0 = ci * chunk
                    xch = xpool.tile([D, chunk, BSH], F32, tag="xch",
                                     name="xch")
                    nc.sync.dma_start(xch[:], X[:, bass.ds(t0, chunk), :])
                    ach = apool.tile([1, chunk * BSH], F32, tag="ach",
                                     name="ach")
                    nc.sync.dma_start(ach[:],
                                      A[0:1, bass.ds(t0 * BSH, chunk * BSH)])
                    h_end = _emit_chunk(nc, pools, consts, hst[:], t0,
                                        xch, ach, OUT, chunk, dyn=True)
                    nc.vector.tensor_copy(hst[:], h_end)
            else:
                hst = constp.tile([D, BSH], F32, tag="hst", name="h_state")
                nc.gpsimd.memset(hst[:], 0.0)
                h_cur = hst[:]
                for ci in range(nchunks):
                    c0 = ci * chunk
                    xch = xpool.tile([D, chunk, BSH], F32, tag="xch",
                                     name=f"xch_{ci}")
                    nc.sync.dma_start(xch[:], X[:, c0 : c0 + chunk, :])
                    ach = apool.tile([1, chunk * BSH], F32, tag="ach",
                                     name=f"ach_{ci}")
                    nc.sync.dma_start(ach[:],
                                      A[0:1, c0 * BSH : (c0 + chunk) * BSH])
                    h_cur = _emit_chunk(nc, pools, consts, h_cur, c0,
                                        xch, ach, OUT, chunk, dyn=False)

    nc.finalize()
    return nc


def build_module(t_steps: int = T, chunk: int = CHUNK, looped: bool = False):
    key = (t_steps, chunk, looped)
    if key in _module_cache:
        return _module_cache[key]
    assert t_steps % chunk == 0
    nc = bacc.Bacc("TRN2", target_bir_lowering=False)
    nc = _build(nc, t_steps, chunk, looped)
    _module_cache[key] = nc
    return nc


def kernel(rnn_input, att_score, gate_kernel, gate_bias, cand_kernel,
           cand_bias, sequence_length, _t_steps: int = T,
           _looped: bool = False):
    """Full-input entry point: shard across 8 cores, run, unshard."""
    t_steps = _t_steps
    rnn_input = np.ascontiguousarray(np.asarray(rnn_input, dtype=np.float32))
    att_score = np.asarray(att_score, dtype=np.float32)
    gate_kernel = np.ascontiguousarray(np.asarray(gate_kernel, dtype=np.float32))
    gate_bias = np.asarray(gate_bias, dtype=np.float32).reshape(2 * D)
    cand_kernel = np.ascontiguousarray(np.asarray(cand_kernel, dtype=np.float32))
    cand_bias = np.asarray(cand_bias, dtype=np.float32).reshape(D)
    lens = np.asarray(sequence_length, dtype=np.int32).reshape(-1)

    nc = build_module(t_steps, CHUNK, _looped)

    in_maps = []
    for cid in range(NCORES):
        sl = slice(cid * BSH, (cid + 1) * BSH)
        xs = rnn_input[sl, :t_steps, :]                         # [BSH, t, D]
        Xc = np.ascontiguousarray(np.transpose(xs, (2, 1, 0)))  # [D, t, BSH]
        al = 1.0 - att_score[sl, :t_steps, 0]                   # [BSH, t]
        Ac = np.ascontiguousarray(al.T).reshape(1, t_steps * BSH)
        in_maps.append({
            "X": Xc, "A": Ac,
            "GK": gate_kernel, "CK": cand_kernel,
            "GBR": np.ascontiguousarray(gate_bias[:D].reshape(D, 1)),
            "GBU": np.ascontiguousarray(gate_bias[D:].reshape(D, 1)),
            "CBC": np.ascontiguousarray(cand_bias.reshape(D, 1)),
        })

    res = run_bass_kernel_spmd(nc, in_maps, list(range(NCORES)))

    out = np.empty((B, t_steps, D), dtype=np.float32)
    for cid in range(NCORES):
        oc = res.results[cid]["OUT"]                            # [t, D, BSH]
        out[cid * BSH : (cid + 1) * BSH] = np.transpose(oc, (2, 0, 1))

    tmask = np.arange(t_steps)[None, :] >= np.minimum(lens, t_steps)[:, None]
    out[tmask] = 0.0
    return out
